# revision 13
# baseline (speedup 1.0000x reference)
"""GCN (2-layer GCNConv + log_softmax) on 8 Trainium2 NeuronCores.

Strategy:
  - Nodes sharded contiguously across 8 cores (12500 each). Layer matmuls
    (x@W1, @W2-delta) run on PE per core.
  - Per-layer scaled features g = dinv * (h@W) are all-gathered into a
    per-core DRAM table (rows padded to 256B so dma_gather can stride).
  - Edge aggregation: per-edge dma_gather (bf16, 32B rows) over dst-sorted,
    degree-class-grouped edge streams; DVE tensor_reduce does the fixed-
    length segment sums; dma_scatter_add (CCE) merges per-bin partial acc
    into a DRAM accumulator initialized with the self-loop term.
  - log_softmax over 2 classes computed as softplus of the logit delta.

All graph preprocessing (sharding, sorting, degree classes, int16 index
streams) happens on host in numpy; all FP math runs on device.
"""

import os
import numpy as np
import ml_dtypes

# ----------------------------------------------------------------------------
# constants
# ----------------------------------------------------------------------------
NCORES = 8
N_FEAT = 256
HID = 16
WIN = int(os.environ.get("GCN_WIN", "32767"))   # int16 index window (rows/bin)
MAX_CLASS = 64       # segments longer than this are split
CHUNK_SLOTS = 64     # msg slots per gather call (64*128 = 8192 idx max)

_bf16 = ml_dtypes.bfloat16

_EXEC_TIME_NS = [None]   # filled when GCN_TRACE=1


def _install_patches():
    import json as _json
    import types
    import concourse.tile as tile
    import concourse.mybir as mybir
    from concourse.vector_clock import ScopedClock

    # --- final drain: one wait per nop (walrus rejects multi-wait insts) ---
    def _drain_and_barrier_split(self, tick_clock, wait_clock):
        nc = self.nc
        anchor = nc.sync.nop(hint="drain_wait_anchor", nofuse=True)
        wait_clock.add_sem_waits(
            anchor.ins, ScopedClock({None: tick_clock.global_clock})
        )
        waits = list(anchor.ins.sync_info.on_wait)
        anchor.ins.sync_info.on_wait = waits[:1]
        for w in waits[1:]:
            nop_inst = nc.sync.nop(hint="drain_wait_split", nofuse=True)
            nop_inst.ins.sync_info = mybir.SyncInfo(on_wait=[w], on_update=[])
        nc.sync.drain()
        nc.all_engine_barrier()
        assert self.sems is not None
        popped = nc._tile_sem_poison_stack.pop()
        assert popped is self._sem_poison
        nc.clear_and_free_semaphores(list(self.sems.allocated().values()))
        nc.all_engine_barrier()

    if getattr(tile.TileContext, "_gcn_patched", False):
        return
    tile.TileContext._drain_and_barrier = _drain_and_barrier_split
    tile.TileContext._gcn_patched = True

    # --- BIR post-pass: hoist excess sync waits onto EventSemaphore nops ---
    ctr = [0]

    def _split_bir_waits(bir_json: bytes) -> bytes:
        d = _json.loads(bir_json)
        changed = False
        for fn in d.get("functions", []):
            for blk in fn.get("blocks", []):
                new_insts = []
                for ins in blk.get("instructions", []):
                    si = ins.get("sync_info")
                    waits = (si or {}).get("on_wait") or []
                    if len(waits) > 1:
                        for w in waits[1:]:
                            ctr[0] += 1
                            new_insts.append({
                                "debug": ins.get("debug", 0),
                                "engine": ins["engine"],
                                "ins": [], "outs": [],
                                "name": f"waitsplit-{ctr[0]}",
                                "opcode": "EventSemaphore",
                                "sync_info": {"on_update": [],
                                              "on_wait": [w]},
                            })
                        si["on_wait"] = waits[:1]
                        changed = True
                    new_insts.append(ins)
                blk["instructions"] = new_insts
        return _json.dumps(d).encode() if changed else bir_json

    import concourse.bass_utils as bass_utils
    import concourse.bass2jax as bass2jax

    orig_compile = bass_utils.compile_bir_kernel

    def compile_split(bir_json, tmpdir, neff_name="file.neff"):
        return orig_compile(_split_bir_waits(bir_json), tmpdir, neff_name)

    bass_utils.compile_bir_kernel = compile_split
    bass2jax.compile_bir_kernel = compile_split
    bass_utils.upload_artifacts = lambda tmpdir: f"file://{tmpdir}"

    # --- NTFF profiling hook (image's antenv lacks axon_hooks) ---
    import sys
    if "antenv.axon_hooks" not in sys.modules:
        mod = types.ModuleType("antenv.axon_hooks")
        hook = [None]
        mod.set_axon_ntff_profile_hook = lambda h: hook.__setitem__(0, h)
        mod.get_axon_ntff_profile_hook = lambda: hook[0]
        sys.modules["antenv.axon_hooks"] = mod
        try:
            import antenv
            antenv.axon_hooks = mod
        except ImportError:
            pass
        try:
            from trn_agent_boot.trn_boot import _ntff_profile_via_ctypes
            so = "/opt/axon/libaxon_pjrt.so"
            if os.path.exists(so):
                mod.set_axon_ntff_profile_hook(_ntff_profile_via_ctypes(so))
        except Exception:
            pass


def _dma_gather_raw(gpsimd, out_ap, in_ap, idxs_ap, num_idxs, elem_size,
                    elem_step):
    """bass dma_gather without the (transpose-only) elem%256B assert."""
    import concourse.mybir as mybir
    from concourse import ap_utils
    from concourse._compat import exact_div
    from concourse.bass import MemorySpace, round_up_to_multiple

    self = gpsimd
    assert idxs_ap.dtype == mybir.dt.int16
    assert in_ap.space == MemorySpace.DRAM
    assert idxs_ap.space == MemorySpace.SBUF
    assert out_ap.space == MemorySpace.SBUF
    assert in_ap.dtype == out_ap.dtype
    assert ap_utils.ap_is_contiguous(out_ap.ap[1:])
    assert ap_utils.ap_is_contiguous(idxs_ap.ap[1:])
    assert in_ap.ap[-1][1] == out_ap.ap[-1][1] == elem_size
    assert out_ap.ap[0][1] * out_ap.ap[1][1] == round_up_to_multiple(num_idxs, 128)
    assert in_ap.ap[0][0] == elem_step
    stride_bytes_256 = exact_div(elem_step * mybir.dt.size(in_ap.dtype), 256)
    assert 0 < stride_bytes_256 < 256

    _in_ap = self.lower_ap_dma(in_ap, for_custom_bir_dma=True)
    _idxs_ap = self.lower_ap(idxs_ap)
    _out_ap = self.lower_ap(out_ap)
    return self.add_instruction(
        mybir.InstDMAGatherAnt(
            name=self.bass.get_next_instruction_name(),
            ins=[*_in_ap, _idxs_ap,
                 self.lower_val_access(self.to_reg(num_idxs))],
            outs=[_out_ap],
            transpose=False, num_idxs=num_idxs, elem_size=elem_size,
            stride_bytes_256=stride_bytes_256, gen_mode=0,
            single_packet=False, queue_num=0,
            sbuf_tokens_per_rank=0, sbuf_free_dim_per_rank=0,
            sbuf_free_dim_pad_per_rank=0, sbuf_byte_offset=0,
        )
    )


# ----------------------------------------------------------------------------
# host-side graph preprocessing
# ----------------------------------------------------------------------------
def _wrap16(stream):
    """Flat int16 stream -> [128, L/16] (idx j <-> partition j%16, col j//16,
    replicated across the 8 Q7 core groups)."""
    L = stream.shape[0]
    assert L % 16 == 0
    arr = stream.reshape(L // 16, 16).T.astype(np.int16)   # [16, cols]
    return np.tile(arr, (8, 1)).copy()


def _prep(edge_index, n_nodes):
    """Build per-core aggregation plan + index streams."""
    npc = n_nodes // NCORES
    npad = ((npc + 127) // 128) * 128
    tiles = npad // 128
    rps = npad + 1                       # rows per shard (+1 zero row)
    tbl_rows = NCORES * rps
    nbins = (tbl_rows + WIN - 1) // WIN

    src = np.asarray(edge_index[0], dtype=np.int64)
    dst = np.asarray(edge_index[1], dtype=np.int64)
    deg = (np.bincount(dst, minlength=n_nodes) + 1).astype(np.float32)

    # node -> (core, r_loc) ; r_loc = (l%128)*tiles + l//128 so that the
    # device's (partition, tile) DMA iteration order is row-sequential.
    def r_of(local):
        return (local % 128) * tiles + local // 128

    c_src = src // npc
    l_src = src - c_src * npc
    row_src = c_src * rps + r_of(l_src)
    bin_src = row_src // WIN
    loc_src = (row_src - bin_src * WIN).astype(np.int16)

    c_dst = dst // npc
    dst_r = r_of(dst - c_dst * npc)

    # one zero row per bin: shard-trailing rows c*rps + npad
    zrows = np.arange(NCORES) * rps + npad
    zloc = np.full(nbins, -1, dtype=np.int64)
    for zr in zrows:
        w = zr // WIN
        if zloc[w] < 0:
            zloc[w] = zr - w * WIN
    assert (zloc >= 0).all(), f"no zero row in some bin: {zloc}"

    trash = npad                          # acc trash row

    # ---- per (core, bin): segment structure ----
    # seg_data[c][w] = dict d -> (locs [n_d, d], dsts [n_d])
    seg_data = [[dict() for _ in range(nbins)] for _ in range(NCORES)]
    order = np.lexsort((loc_src, dst_r, bin_src, c_dst))
    cs, bs, ds_, ls = c_dst[order], bin_src[order], dst_r[order], loc_src[order]
    # boundaries of (core, bin, dst) groups
    key = (cs * nbins + bs) * npad + ds_
    starts = np.flatnonzero(np.r_[True, key[1:] != key[:-1]])
    counts = np.diff(np.r_[starts, key.shape[0]])
    g_core = cs[starts]
    g_bin = bs[starts]
    g_dst = ds_[starts]
    for c in range(NCORES):
        for w in range(nbins):
            m = (g_core == c) & (g_bin == w)
            if not m.any():
                continue
            st, ct, dr = starts[m], counts[m], g_dst[m]
            assert ct.max() <= MAX_CLASS, f"degree class {ct.max()} > {MAX_CLASS}"
            for d in np.unique(ct):
                mm = ct == d
                idx2 = st[mm][:, None] + np.arange(d)[None, :]
                seg_data[c][w][int(d)] = (ls[idx2], dr[mm])

    # ---- common (max-padded) structure per bin ----
    plan_bins = []
    gidx_all = [[] for _ in range(NCORES)]
    sidx_all = [[] for _ in range(NCORES)]
    for w in range(nbins):
        classes = sorted({d for c in range(NCORES) for d in seg_data[c][w]})
        S = {d: max((seg_data[c][w][d][0].shape[0] if d in seg_data[c][w]
                     else 0) for c in range(NCORES)) for d in classes}
        S = {d: (S[d] + 127) // 128 for d in classes}     # rounds of 128 segs
        # piece slot offsets per class
        poff, T = {}, 0
        for d in classes:
            poff[d] = T
            T += S[d]
        # gather calls: walk classes/rounds, cut at <= CHUNK_SLOTS slots
        calls = []
        cur = {"slots": 0, "reduces": []}
        for d in classes:
            s = 0
            while s < S[d]:
                if cur["slots"] + d > CHUNK_SLOTS:
                    calls.append(cur)
                    cur = {"slots": 0, "reduces": []}
                take = min((CHUNK_SLOTS - cur["slots"]) // d, S[d] - s)
                cur["reduces"].append(
                    dict(off=cur["slots"], S=take, d=d, poff=poff[d] + s))
                cur["slots"] += take * d
                s += take
        if cur["slots"]:
            calls.append(cur)
        # scatter calls: piece slots chunked
        scalls = []
        a = 0
        while a < T:
            b = min(a + 32, T)          # scatter_add caps out below 8192 idx
            scalls.append((a, b))
            a = b
        plan_bins.append(dict(classes=classes, S=S, poff=poff, T=T,
                              calls=calls, scalls=scalls,
                              gcols=sum(c["slots"] for c in calls) * 8,
                              scols=T * 8))

        # ---- per-core index streams ----
        for c in range(NCORES):
            gparts, sparts = [], []
            for d in classes:
                nrounds = S[d]
                locs, drs = seg_data[c][w].get(d, (np.zeros((0, d), np.int16),
                                                  np.zeros(0, np.int64)))
                n = locs.shape[0]
                N_d = nrounds * 128
                locs_p = np.full((N_d, d), zloc[w], dtype=np.int16)
                locs_p[:n] = locs
                dst_p = np.full(N_d, trash, dtype=np.int64)
                dst_p[:n] = drs
                # seg m -> (p=m%128, s=m//128); row ((s*d)+r)*128+p
                g = locs_p.reshape(nrounds, 128, d).transpose(0, 2, 1)
                gparts.append(g.reshape(-1))
                sparts.append(dst_p.reshape(nrounds, 128).reshape(-1))
            gidx_all[c].append(_wrap16(np.concatenate(gparts)
                                       if gparts else np.zeros(0, np.int16)))
            sidx_all[c].append(_wrap16(np.concatenate(sparts).astype(np.int16)
                                       if sparts else np.zeros(0, np.int16)))

    plan = dict(npc=npc, npad=npad, tiles=tiles, rps=rps,
                tbl_rows=tbl_rows, nbins=nbins, trash=trash,
                bins=plan_bins, zrows=zrows)
    return plan, gidx_all, sidx_all, deg


# ----------------------------------------------------------------------------
# bass kernel builder
# ----------------------------------------------------------------------------
def _build_bass(plan):
    import concourse.bass as bass
    import concourse.bacc as bacc
    import concourse.mybir as mybir
    import concourse.tile as tile

    npad, tiles, rps = plan["npad"], plan["tiles"], plan["rps"]
    tbl_rows, nbins = plan["tbl_rows"], plan["nbins"]
    acc_rows = npad + 1
    f32, bf16, i16 = mybir.dt.float32, mybir.dt.bfloat16, mybir.dt.int16
    AP = bass.AP

    nc = bacc.Bacc(None, target_bir_lowering=False)

    xT = nc.declare_dram_parameter("xT", [128, 2 * npad], bf16, isOutput=False)
    w1 = nc.declare_dram_parameter("w1", [128, 32], bf16, isOutput=False)
    degp = nc.declare_dram_parameter("degp", [128, tiles], f32, isOutput=False)
    b1v = nc.declare_dram_parameter("b1v", [128, 16], f32, isOutput=False)
    wd = nc.declare_dram_parameter("wd", [128, 16], f32, isOutput=False)
    bd = nc.declare_dram_parameter("bd", [128, 1], f32, isOutput=False)
    gidx_p = [nc.declare_dram_parameter(f"gidx{w}", [128, max(plan["bins"][w]["gcols"], 16)],
                                        i16, isOutput=False) for w in range(nbins)]
    sidx_p = [nc.declare_dram_parameter(f"sidx{w}", [128, max(plan["bins"][w]["scols"], 16)],
                                        i16, isOutput=False) for w in range(nbins)]
    out_p = nc.declare_dram_parameter("out", [npad, 2], f32, isOutput=True)

    tables = [nc.dram_tensor(f"table{l}", [tbl_rows, 128], bf16) for l in (0, 1)]
    agins = [nc.dram_tensor(f"agin{l}", [rps, 128], bf16) for l in (0, 1)]
    accs = [nc.dram_tensor(f"acc{l}", [acc_rows, 64], f32) for l in (0, 1)]

    def view(ap, dims, extra_off=0):
        return AP(ap.tensor, ap.offset + extra_off, dims)

    with tile.TileContext(nc) as tc:
        with tc.tile_pool(name="sb", bufs=1) as P1, \
             tc.tile_pool(name="dbl", bufs=2) as P2, \
             tc.tile_pool(name="ps", bufs=2, space="PSUM") as PP:

            # --- constants in ---
            w1_t = P1.tile([128, 32], bf16)
            nc.sync.dma_start(out=w1_t[:], in_=w1[:])
            b1_t = P1.tile([128, 16], f32)
            nc.sync.dma_start(out=b1_t[:], in_=b1v[:])
            wd_t = P1.tile([128, 16], f32)
            nc.sync.dma_start(out=wd_t[:], in_=wd[:])
            bd_t = P1.tile([128, 1], f32)
            nc.sync.dma_start(out=bd_t[:], in_=bd[:])
            deg_t = P1.tile([128, tiles], f32)
            nc.sync.dma_start(out=deg_t[:], in_=degp[:])
            dinv = P1.tile([128, tiles], f32)
            nc.scalar.activation(dinv[:], deg_t[:],
                                 mybir.ActivationFunctionType.Ln)
            nc.scalar.activation(dinv[:], dinv[:],
                                 mybir.ActivationFunctionType.Exp,
                                 bias=0.0, scale=-0.5)

            zero_bf = P1.tile([1, 128], bf16)
            nc.vector.memset(zero_bf[:], 0.0)

            # --- x @ W1 -> g1 = dinv * hw ---
            g1 = P1.tile([128, tiles, 16], f32)
            xk = P1.tile([128, npad], bf16)
            for kc in (0, 1):
                nc.sync.dma_start(out=xk[:], in_=xT[:, kc * npad:(kc + 1) * npad])
                for t in range(tiles):
                    ps = PP.tile([128, 16], f32, tag="mm")
                    nc.tensor.matmul(out=ps[:],
                                     lhsT=xk[:, t * 128:(t + 1) * 128],
                                     rhs=w1_t[:, kc * 16:(kc + 1) * 16],
                                     start=True, stop=True)
                    if kc == 0:
                        nc.vector.tensor_copy(out=g1[:, t, :], in_=ps[:])
                    else:
                        nc.vector.tensor_add(out=g1[:, t, :], in0=g1[:, t, :],
                                             in1=ps[:])
            dinv_b = dinv[:, :].unsqueeze(-1).to_broadcast([128, tiles, 16])
            nc.vector.tensor_mul(out=g1[:, :, :], in0=g1[:, :, :], in1=dinv_b)

            stage = P1.tile([128, tiles, 128], bf16)
            nc.vector.memset(stage[:], 0.0)
            accst = P1.tile([128, tiles, 64], f32)
            nc.vector.memset(accst[:], 0.0)

            gsrc = g1
            for layer in (0, 1):
                table, agin, acc = tables[layer], agins[layer], accs[layer]
                binfo = plan["bins"]

                # table build: padded shard -> allgather
                nc.vector.tensor_copy(out=stage[:, :, :16], in_=gsrc[:, :, :])
                nc.sync.dma_start(out=agin[:npad, :], in_=stage[:, :, :])
                nc.sync.dma_start(out=agin[npad:npad + 1, :], in_=zero_bf[:])
                nc.gpsimd.collective_compute(
                    "AllGather", mybir.AluOpType.bypass,
                    replica_groups=[list(range(NCORES))],
                    ins=[agin[:, :].opt()], outs=[table[:, :].opt()])

                # acc init = self-loop term
                nc.vector.tensor_copy(out=accst[:, :, :16], in_=gsrc[:, :, :])
                nc.sync.dma_start(out=acc[:npad, :], in_=accst[:, :, :])

                # aggregation
                for w in range(nbins):
                    bi = binfo[w]
                    if bi["T"] == 0:
                        continue
                    gi = P2.tile([128, max(bi["gcols"], 16)], i16, tag="gi")
                    nc.sync.dma_start(out=gi[:], in_=gidx_p[w][:])
                    si = P2.tile([128, max(bi["scols"], 16)], i16, tag="si")
                    nc.sync.dma_start(out=si[:], in_=sidx_p[w][:])
                    piece = P2.tile([128, bi["T"], 16], f32, tag="piece")
                    win_lo = w * WIN
                    win_n = min(WIN, tbl_rows - win_lo)
                    tbl_win = table[win_lo:win_lo + win_n, :16]
                    col = 0
                    for call in bi["calls"]:
                        slots = call["slots"]
                        rows = slots * 128
                        msg = P2.tile([128, CHUNK_SLOTS, 16], bf16, tag="msg")
                        _dma_gather_raw(
                            nc.gpsimd,
                            out_ap=msg[:, :slots, :],
                            in_ap=tbl_win,
                            idxs_ap=gi[:, col:col + slots * 8],
                            num_idxs=rows, elem_size=16, elem_step=128)
                        col += slots * 8
                        base = msg[:, :, :]
                        for r in call["reduces"]:
                            rv = view(base,
                                      [base.ap[0],
                                       (r["d"] * 16, r["S"]),
                                       (1, 16),
                                       (16, r["d"])],
                                      extra_off=r["off"] * 16)
                            nc.vector.tensor_reduce(
                                out=piece[:, r["poff"]:r["poff"] + r["S"], :],
                                in_=rv, axis=mybir.AxisListType.X,
                                op=mybir.AluOpType.add)
                    for (a, b) in bi["scalls"]:
                        nc.gpsimd.dma_scatter_add(
                            out_ap=acc[:, :16],
                            in_ap=piece[:, a:b, :],
                            idxs_ap=si[:, a * 8:b * 8],
                            num_idxs=(b - a) * 128,
                            num_idxs_reg=(b - a) * 128,
                            elem_size=16, elem_step=64, single_packet=False)

                # epilogue: readback + pointwise
                rb = P1.tile([128, tiles, 64], f32, tag="rb")
                nc.sync.dma_start(out=rb[:, :, :], in_=acc[:npad, :])
                q = P1.tile([128, tiles, 16], f32, tag="q")
                nc.vector.tensor_mul(out=q[:, :, :], in0=rb[:, :, :16],
                                     in1=dinv_b)
                if layer == 0:
                    b1_b = b1_t[:, :].unsqueeze(1).to_broadcast([128, tiles, 16])
                    nc.vector.tensor_add(out=q[:, :, :], in0=q[:, :, :],
                                         in1=b1_b)
                    h = P1.tile([128, tiles, 16], f32, tag="scr16")
                    nc.vector.tensor_scalar(out=h[:, :, :], in0=q[:, :, :],
                                            scalar1=0.0, scalar2=None,
                                            op0=mybir.AluOpType.max)
                    g2 = P1.tile([128, tiles, 16], f32)
                    nc.vector.tensor_mul(out=g2[:, :, :], in0=h[:, :, :],
                                         in1=dinv_b)
                    gsrc = g2
                else:
                    wd_b = wd_t[:, :].unsqueeze(1).to_broadcast([128, tiles, 16])
                    tmp = P1.tile([128, tiles, 16], f32, tag="scr16")
                    nc.vector.tensor_mul(out=tmp[:, :, :], in0=q[:, :, :],
                                         in1=wd_b)
                    delta = P1.tile([128, tiles, 1], f32)
                    nc.vector.tensor_reduce(out=delta[:, :, :], in_=tmp[:, :, :],
                                            axis=mybir.AxisListType.X,
                                            op=mybir.AluOpType.add)
                    bd_b = bd_t[:, :].unsqueeze(1).to_broadcast([128, tiles, 1])
                    nc.vector.tensor_add(out=delta[:, :, :], in0=delta[:, :, :],
                                         in1=bd_b)
                    # softplus(d) = m + ln(exp(-m) + exp(d-m)), m = max(d, 0)
                    m_t = P1.tile([128, tiles, 1], f32)
                    nc.vector.tensor_scalar(out=m_t[:, :, :],
                                            in0=delta[:, :, :],
                                            scalar1=0.0, scalar2=None,
                                            op0=mybir.AluOpType.max)
                    e1 = P1.tile([128, tiles, 1], f32)
                    nc.vector.tensor_sub(out=e1[:, :, :], in0=delta[:, :, :],
                                         in1=m_t[:, :, :])
                    nc.scalar.activation(e1[:, :, :], e1[:, :, :],
                                         mybir.ActivationFunctionType.Exp)
                    e2 = P1.tile([128, tiles, 1], f32)
                    nc.scalar.activation(e2[:, :, :], m_t[:, :, :],
                                         mybir.ActivationFunctionType.Exp,
                                         bias=0.0, scale=-1.0)
                    sp = P1.tile([128, tiles, 1], f32)
                    nc.vector.tensor_add(out=sp[:, :, :], in0=e1[:, :, :],
                                         in1=e2[:, :, :])
                    nc.scalar.activation(sp[:, :, :], sp[:, :, :],
                                         mybir.ActivationFunctionType.Ln)
                    nc.vector.tensor_add(out=sp[:, :, :], in0=sp[:, :, :],
                                         in1=m_t[:, :, :])
                    outt = P1.tile([128, tiles, 2], f32)
                    nc.vector.tensor_scalar(out=outt[:, :, 0:1], in0=sp[:, :, :],
                                            scalar1=-1.0, scalar2=None,
                                            op0=mybir.AluOpType.mult)
                    nc.vector.tensor_sub(out=outt[:, :, 1:2], in0=delta[:, :, :],
                                         in1=sp[:, :, :])
                    nc.sync.dma_start(out=out_p[:, :], in_=outt[:, :, :])

    nc.finalize()
    return nc


# ----------------------------------------------------------------------------
# public entry
# ----------------------------------------------------------------------------
_CACHE = {}


def kernel(x, edge_index, W1, b1, W2, b2):
    _install_patches()
    from concourse.bass_utils import run_bass_kernel_spmd

    n = x.shape[0]
    plan, gidx_all, sidx_all, deg = _prep(edge_index, n)
    npc, npad, tiles = plan["npc"], plan["npad"], plan["tiles"]

    key = (n, tuple(tuple((b["gcols"], b["scols"],
                           tuple(c["slots"] for c in b["calls"]))
                          for b in plan["bins"])))
    if key not in _CACHE:
        _CACHE.clear()
        _CACHE[key] = _build_bass(plan)
    nc = _CACHE[key]

    wdiff = (W2[:, 1] - W2[:, 0]).astype(np.float32)
    bdiff = np.float32(b2[1] - b2[0])

    in_maps = []
    for c in range(NCORES):
        xc = np.zeros((npad, N_FEAT), np.float32)
        xc[:npc] = x[c * npc:(c + 1) * npc]
        # xT[p, kc*npad + j] = xc[j, kc*128+p]
        xT = np.ascontiguousarray(
            xc.T.reshape(2, 128, npad).transpose(1, 0, 2).reshape(128, 2 * npad)
        ).astype(_bf16)
        degc = np.ones(npad, np.float32)
        degc[:npc] = deg[c * npc:(c + 1) * npc]
        degp = np.ascontiguousarray(degc.reshape(tiles, 128).T)
        w1p = np.ascontiguousarray(
            W1.astype(np.float32).reshape(2, 128, 16).transpose(1, 0, 2)
            .reshape(128, 32)).astype(_bf16)
        m = dict(xT=xT, w1=w1p, degp=degp,
                 b1v=np.tile(b1.reshape(1, 16).astype(np.float32), (128, 1)),
                 wd=np.tile(wdiff.reshape(1, 16), (128, 1)),
                 bd=np.full((128, 1), bdiff, np.float32))
        for w in range(plan["nbins"]):
            g = gidx_all[c][w]
            s = sidx_all[c][w]
            if g.shape[1] == 0:
                g = np.zeros((128, 16), np.int16)
            if s.shape[1] == 0:
                s = np.zeros((128, 16), np.int16)
            m[f"gidx{w}"] = np.ascontiguousarray(g)
            m[f"sidx{w}"] = np.ascontiguousarray(s)
        in_maps.append(m)

    trace = bool(int(os.environ.get("GCN_TRACE", "0")))
    res = run_bass_kernel_spmd(nc, in_maps, core_ids=list(range(NCORES)),
                               trace=trace)
    _EXEC_TIME_NS[0] = res.exec_time_ns

    out = np.empty((n, 2), np.float32)
    l = np.arange(npc)
    r_loc = (l % 128) * tiles + l // 128
    for c in range(NCORES):
        out[c * npc:(c + 1) * npc] = res.results[c]["out"][r_loc]
    return out
